# revision 1
# baseline (speedup 1.0000x reference)
"""Single-head causal attention on 8 NeuronCores (batch-parallel).

x [8, 2048, 1024], Wq/Wk/Wv [1024, 64] -> out [8, 2048, 64].
Each core handles one batch element:
  qkT = [Wq|Wk].T @ x.T        (PE, contraction over C, M=128 combined)
  vT  = Wv.T @ x.T
  weiT[s,t] = k[s]·q[t]        (scores in transposed layout)
  pT = exp(weiT/sqrt(H))       (no max-subtraction: |scores| <~ 6)
  outT_aug = [v|1].T @ pT      (ones column yields softmax denominators)
  out[t,h] = outT_aug[h,t] / outT_aug[64,t]
x.T is built on-chip with PE transposes. Causality via tile skipping,
column-restricted diagonal matmuls, and one [128,128] triangular mask.

Matmul operands are float32r (single-pass PE fast path). The BIR verifier
requires fp32r matmul inputs to be produced rounded, so every SBUF tile the
PE consumes is declared float32r and written by DVE/ACT/DMA accordingly;
gpsimd mask builders can't write f32r, so masks stage through f32 scratch.
"""

from contextlib import ExitStack

import numpy as np

import concourse.bass as bass
import concourse.mybir as mybir
import concourse.tile as tile
from concourse import bacc
from concourse.bass_utils import run_bass_kernel_spmd
from concourse.masks import make_identity, make_upper_triangular

B, T, C, H = 8, 2048, 1024, 64
P = 128                      # partition tile
NT = T // P                  # 16 row tiles
NC = C // P                  # 8 contraction tiles
CH = 512                     # t-chunk width (psum bank)
NCH = T // CH                # 4 chunks
TPC = CH // P                # 4 t-tiles per chunk
VA = 96                      # padded [v | 1 | 0] width (transposes need 32-align)

MM_DT = mybir.dt.float32r   # PE operand dtype (fp32 bits, single-pass path)
F32 = mybir.dt.float32

Exp = mybir.ActivationFunctionType.Exp


def build_kernel():
    nc = bacc.Bacc(
        "TRN2",
        target_bir_lowering=False,
        debug=False,
        enable_asserts=False,
        num_devices=B,
    )
    xd = nc.dram_tensor("x", [T, C], MM_DT, kind="ExternalInput").ap()
    wqd = nc.dram_tensor("Wq", [C, H], MM_DT, kind="ExternalInput").ap()
    wkd = nc.dram_tensor("Wk", [C, H], MM_DT, kind="ExternalInput").ap()
    wvd = nc.dram_tensor("Wv", [C, H], MM_DT, kind="ExternalInput").ap()
    outd = nc.dram_tensor("out", [T, H], F32, kind="ExternalOutput").ap()

    with tile.TileContext(nc) as tc, ExitStack() as ctx:
        const = ctx.enter_context(tc.tile_pool(name="const", bufs=1))
        persist = ctx.enter_context(tc.tile_pool(name="persist", bufs=1))
        stage_p = ctx.enter_context(tc.tile_pool(name="stage", bufs=3))
        pt_p = ctx.enter_context(tc.tile_pool(name="pt", bufs=4))
        osb_p = ctx.enter_context(tc.tile_pool(name="osb", bufs=2))
        ost_p = ctx.enter_context(tc.tile_pool(name="ost", bufs=2))
        rc_p = ctx.enter_context(tc.tile_pool(name="rc", bufs=8))
        big_ps = ctx.enter_context(tc.tile_pool(name="bigps", bufs=4, space="PSUM"))
        o_ps_p = ctx.enter_context(tc.tile_pool(name="ops", bufs=2, space="PSUM"))
        tp_ps = ctx.enter_context(tc.tile_pool(name="tpps", bufs=2, space="PSUM"))

        # masks: gpsimd builders write f32; DVE copy rounds into f32r
        scr_i = const.tile([P, P], F32, tag="scr_i")
        make_identity(nc, scr_i)
        ident = const.tile([P, P], MM_DT, tag="ident")
        nc.vector.tensor_copy(ident, scr_i)
        scr_t = const.tile([P, P], F32, tag="scr_t")
        make_upper_triangular(nc, scr_t, val=1.0, diag=True)
        tri = const.tile([P, P], MM_DT, tag="tri")  # tri[p,j]=1 iff j>=p
        nc.vector.tensor_copy(tri, scr_t)

        wqk = const.tile([P, NC, P], MM_DT, tag="wqk")  # [Wq|Wk] per c-tile
        nc.sync.dma_start(wqk[:, :, 0:H], wqd.rearrange("(c p) h -> p c h", p=P))
        nc.sync.dma_start(wqk[:, :, H:P], wkd.rearrange("(c p) h -> p c h", p=P))
        wv = const.tile([P, NC, H], MM_DT, tag="wv")
        nc.sync.dma_start(wv, wvd.rearrange("(c p) h -> p c h", p=P))

        xT = persist.tile([P, NC, T], MM_DT, tag="xT")      # x.T: [c, t]
        qkT = persist.tile([P, T], MM_DT, tag="qkT")        # qT rows 0:64, kT 64:128
        kTlo = persist.tile([H, T], MM_DT, tag="kTlo")      # kT at partitions 0:64
        vT = persist.tile([H, T], MM_DT, tag="vT")
        vaug = persist.tile([P, NT, VA], MM_DT, tag="vaug")  # [v | 1 | 0] per s-tile
        ones = nc.const_aps.scalar_like(1.0, vaug)
        nc.vector.tensor_copy(vaug[:, :, H : H + 1], ones.broadcast_to((P, NT, 1)))
        zeros = nc.const_aps.scalar_like(0.0, vaug)
        nc.vector.tensor_copy(
            vaug[:, :, H + 1 : VA], zeros.broadcast_to((P, NT, VA - H - 1))
        )

        for ch in range(NCH):
            chs = slice(ch * CH, (ch + 1) * CH)

            # ---- load x rows for this chunk, transpose into xT ----
            for g in (2 * ch, 2 * ch + 1):  # 2 t-tiles per DMA (1 MB)
                stg = stage_p.tile([P, 2, C], MM_DT)
                nc.sync.dma_start(
                    stg, xd[g * 256 : (g + 1) * 256, :].rearrange("(n p) c -> p n c", p=P)
                )
                for n in range(2):
                    tau = 2 * g + n
                    for hf in range(2):  # c-tiles 4*hf .. 4*hf+3
                        tr = big_ps.tile([P, CH], MM_DT, tag="big")
                        for cc in range(4):
                            c = 4 * hf + cc
                            nc.tensor.transpose(
                                tr[:, cc * P : (cc + 1) * P],
                                stg[:, n, c * P : (c + 1) * P],
                                ident,
                            )
                        dst = xT[:, 4 * hf : 4 * hf + 4, tau * P : (tau + 1) * P]
                        src_ap = tr.rearrange("p (c t) -> p c t", c=4)
                        if ch % 2 == 0:
                            nc.vector.tensor_copy(dst, src_ap)
                        else:
                            nc.scalar.copy(dst, src_ap)

            # ---- projections for this chunk's t-columns ----
            qk_ps = big_ps.tile([P, CH], F32, tag="big")
            for c in range(NC):
                nc.tensor.matmul(
                    qk_ps, wqk[:, c, :], xT[:, c, chs], start=(c == 0), stop=(c == NC - 1)
                )
            nc.scalar.copy(qkT[:, chs], qk_ps)
            nc.sync.dma_start(kTlo[:, chs], qkT[H:P, chs])

            v_ps = o_ps_p.tile([H, CH], F32, tag="o")
            for c in range(NC):
                nc.tensor.matmul(
                    v_ps, wv[:, c, :], xT[:, c, chs], start=(c == 0), stop=(c == NC - 1)
                )
            nc.vector.tensor_copy(vT[:, chs], v_ps)
            vt_ps = tp_ps.tile([P, TPC * H], MM_DT, tag="tp")
            for j in range(TPC):
                s = TPC * ch + j
                nc.tensor.transpose(
                    vt_ps[:, j * H : (j + 1) * H],
                    vT[:, s * P : (s + 1) * P],
                    ident[0:H, 0:H],
                )
            nc.vector.tensor_copy(
                vaug[:, TPC * ch : TPC * ch + TPC, 0:H],
                vt_ps.rearrange("p (j h) -> p j h", j=TPC),
            )

            # ---- attention: scores vs all causal key tiles, exp, PV ----
            smax = TPC * ch + TPC - 1
            o_ps = o_ps_p.tile([VA, CH], F32, tag="o")
            prev = None
            for s in range(smax + 1):
                diag = s >= TPC * ch
                col0 = (s - TPC * ch) * P if diag else 0
                wei = big_ps.tile([P, CH], F32, tag="big")
                nc.tensor.matmul(
                    wei[:, col0:],
                    kTlo[:, s * P : (s + 1) * P],
                    qkT[0:H, ch * CH + col0 : (ch + 1) * CH],
                    start=True,
                    stop=True,
                )
                pT = pt_p.tile([P, CH], MM_DT)
                nc.scalar.activation(pT[:, col0:], wei[:, col0:], Exp, scale=float(H) ** -0.5)
                if diag:
                    nc.vector.tensor_mul(
                        pT[:, col0 : col0 + P], pT[:, col0 : col0 + P], tri
                    )
                if prev is not None:
                    pcol0, ppT, ps = prev
                    nc.tensor.matmul(
                        o_ps[:, pcol0:], vaug[:, ps, :], ppT[:, pcol0:],
                        start=(ps == 0), stop=False,
                    )
                prev = (col0, pT, s)
            pcol0, ppT, ps = prev
            nc.tensor.matmul(
                o_ps[:, pcol0:], vaug[:, ps, :], ppT[:, pcol0:],
                start=(ps == 0), stop=True,
            )

            # ---- epilogue: transpose back, normalize, store ----
            osb = osb_p.tile([VA, CH], MM_DT)
            nc.scalar.copy(osb, o_ps)
            ot_ps = tp_ps.tile([P, TPC * VA], MM_DT, tag="tp")
            for j in range(TPC):
                nc.tensor.transpose(
                    ot_ps[:, j * VA : (j + 1) * VA],
                    osb[:, j * P : (j + 1) * P],
                    ident[0:VA, 0:VA],
                )
            ost = ost_p.tile([P, TPC, H], F32)
            for j in range(TPC):
                rc = rc_p.tile([P, 1], F32)
                nc.vector.reciprocal(rc, ot_ps[:, j * VA + H : j * VA + H + 1])
                nc.vector.tensor_scalar_mul(
                    ost[:, j, :], ot_ps[:, j * VA : j * VA + H], rc
                )
            nc.sync.dma_start(
                outd[ch * CH : (ch + 1) * CH, :].rearrange("(n p) h -> p n h", p=P), ost
            )

    nc.compile()
    return nc


_NC = None


def kernel(x, Wq, Wk, Wv, **run_kwargs):
    global _NC
    if _NC is None:
        _NC = build_kernel()
    x = np.ascontiguousarray(np.asarray(x, dtype=np.float32))
    Wq = np.ascontiguousarray(np.asarray(Wq, dtype=np.float32))
    Wk = np.ascontiguousarray(np.asarray(Wk, dtype=np.float32))
    Wv = np.ascontiguousarray(np.asarray(Wv, dtype=np.float32))
    in_maps = [
        {"x": x[b], "Wq": Wq, "Wk": Wk, "Wv": Wv} for b in range(B)
    ]
    res = run_bass_kernel_spmd(_NC, in_maps, core_ids=list(range(B)), **run_kwargs)
    out = np.stack([res.results[b]["out"] for b in range(B)])
    if run_kwargs:
        kernel.last_result = res
    return out


if __name__ == "__main__":
    rng = np.random.default_rng(0)
    ins = {
        "x": rng.standard_normal((B, T, C), dtype=np.float32),
        "Wq": rng.standard_normal((C, H), dtype=np.float32) / np.sqrt(C),
        "Wk": rng.standard_normal((C, H), dtype=np.float32) / np.sqrt(C),
        "Wv": rng.standard_normal((C, H), dtype=np.float32) / np.sqrt(C),
    }
    out = kernel(**ins)
    print("out", out.shape, out.dtype)



# revision 2
# speedup vs baseline: 1.4544x; 1.4544x over previous
"""Single-head causal attention on 8 NeuronCores (batch-parallel).

x [8, 2048, 1024], Wq/Wk/Wv [1024, 64] -> out [8, 2048, 64].
Each core handles one batch element.

v2: host-side layout prep removes all on-chip x transposes.
  - x is uploaded pre-transposed (xT [C, T]) and cast to bf16 on host,
    so projections consume it directly as the moving operand; no PE
    transposes of x, no PSUM->SBUF staging copies, half the DMA bytes.
  - [Wq|Wk] and Wv are host-packed into per-c-tile stationary layouts
    (contiguous 2KB/partition DMA descriptors instead of 256B rows).
  - All PE operands are bf16 (FWL weight loads); accumulation stays f32
    in PSUM.  exp() runs on ACT only; every copy runs on DVE.
  qkT = [Wq|Wk].T @ xT      (per 512-col chunk; q rows 0:64, k 64:128)
  vT  = Wv.T @ xT
  weiT[s,t] = k[s]·q[t]     (scores transposed; K=64 matmul)
  pT  = exp(weiT/8)         (no max-subtraction: |scores/8| <~ 6)
  outT_aug = [v|1].T @ pT   (ones column yields softmax denominators)
  out[t,h] = outT_aug[h,t] / outT_aug[64,t]
Causality via tile skipping, column-restricted diagonal matmuls, and one
[128,128] triangular mask.
"""

from contextlib import ExitStack

import numpy as np
import ml_dtypes

import concourse.bass as bass
import concourse.mybir as mybir
import concourse.tile as tile
from concourse import bacc
from concourse.bass_utils import run_bass_kernel_spmd
from concourse.masks import make_identity, make_upper_triangular

B, T, C, H = 8, 2048, 1024, 64
P = 128                      # partition tile
NT = T // P                  # 16 row tiles
NC = C // P                  # 8 contraction tiles
CH = 512                     # t-chunk width (psum bank)
NCH = T // CH                # 4 chunks
TPC = CH // P                # 4 t-tiles per chunk
VA = 96                      # padded [v | 1 | 0] width (transposes need 32-align)

BF16 = mybir.dt.bfloat16
F32 = mybir.dt.float32

Exp = mybir.ActivationFunctionType.Exp

BF16_NP = ml_dtypes.bfloat16


def build_kernel():
    nc = bacc.Bacc(
        "TRN2",
        target_bir_lowering=False,
        debug=False,
        enable_asserts=False,
        num_devices=B,
    )
    xtd = nc.dram_tensor("xT", [C, T], BF16, kind="ExternalInput").ap()
    wqkd = nc.dram_tensor("wqk", [P, NC, P], BF16, kind="ExternalInput").ap()
    wvd = nc.dram_tensor("wv", [P, NC, H], BF16, kind="ExternalInput").ap()
    outd = nc.dram_tensor("out", [T, H], F32, kind="ExternalOutput").ap()

    with tile.TileContext(nc) as tc, ExitStack() as ctx:
        const = ctx.enter_context(tc.tile_pool(name="const", bufs=1))
        persist = ctx.enter_context(tc.tile_pool(name="persist", bufs=1))
        vtmp_p = ctx.enter_context(tc.tile_pool(name="vtmp", bufs=2))
        pt_p = ctx.enter_context(tc.tile_pool(name="pt", bufs=4))
        osb_p = ctx.enter_context(tc.tile_pool(name="osb", bufs=2))
        ost_p = ctx.enter_context(tc.tile_pool(name="ost", bufs=2))
        rc_p = ctx.enter_context(tc.tile_pool(name="rc", bufs=8))
        pj_ps = ctx.enter_context(tc.tile_pool(name="pjps", bufs=2, space="PSUM"))
        wei_ps = ctx.enter_context(tc.tile_pool(name="weips", bufs=3, space="PSUM"))
        o_ps_p = ctx.enter_context(tc.tile_pool(name="ops", bufs=2, space="PSUM"))
        tp_ps = ctx.enter_context(tc.tile_pool(name="tpps", bufs=1, space="PSUM"))

        # weights first (small, few descriptors), then x in three slabs so
        # chunk-0 compute starts as early as possible
        wqk = const.tile([P, NC, P], BF16, tag="wqk")
        nc.sync.dma_start(wqk, wqkd)
        wv = const.tile([P, NC, H], BF16, tag="wv")
        nc.sync.dma_start(wv, wvd)

        xTs = persist.tile([P, NC, T], BF16, tag="xTs")  # x.T: [c, t]
        xsrc = xtd.rearrange("(c p) t -> p c t", p=P)
        nc.sync.dma_start(xTs[:, :, 0:CH], xsrc[:, :, 0:CH])
        nc.sync.dma_start(xTs[:, :, CH : 2 * CH], xsrc[:, :, CH : 2 * CH])
        nc.sync.dma_start(xTs[:, :, 2 * CH : T], xsrc[:, :, 2 * CH : T])

        # masks: gpsimd builders write f32; DVE copy casts to bf16
        scr_i = const.tile([P, P], F32, tag="scr_i")
        make_identity(nc, scr_i)
        ident = const.tile([P, P], BF16, tag="ident")
        nc.vector.tensor_copy(ident, scr_i)
        scr_t = const.tile([P, P], F32, tag="scr_t")
        make_upper_triangular(nc, scr_t, val=1.0, diag=True)
        tri = const.tile([P, P], BF16, tag="tri")  # tri[p,j]=1 iff j>=p
        nc.vector.tensor_copy(tri, scr_t)

        qkT = persist.tile([P, T], BF16, tag="qkT")  # qT rows 0:64, kT 64:128
        kTlo = persist.tile([H, T], BF16, tag="kTlo")  # kT at partitions 0:64
        vaug = persist.tile([P, NT, VA], BF16, tag="vaug")  # [v | 1 | 0] per s-tile
        ones = nc.const_aps.scalar_like(1.0, vaug)
        nc.vector.tensor_copy(vaug[:, :, H : H + 1], ones.broadcast_to((P, NT, 1)))
        zeros = nc.const_aps.scalar_like(0.0, vaug)
        nc.vector.tensor_copy(
            vaug[:, :, H + 1 : VA], zeros.broadcast_to((P, NT, VA - H - 1))
        )

        for ch in range(NCH):
            chs = slice(ch * CH, (ch + 1) * CH)

            # ---- projections for this chunk's t-columns ----
            qk_ps = pj_ps.tile([P, CH], F32, tag="pj")
            for c in range(NC):
                nc.tensor.matmul(
                    qk_ps, wqk[:, c, :], xTs[:, c, chs], start=(c == 0), stop=(c == NC - 1)
                )
            nc.vector.tensor_copy(qkT[:, chs], qk_ps)
            nc.sync.dma_start(kTlo[:, chs], qkT[H:P, chs])

            v_ps = pj_ps.tile([P, CH], F32, tag="pj")
            for c in range(NC):
                nc.tensor.matmul(
                    v_ps[0:H, :], wv[:, c, :], xTs[:, c, chs],
                    start=(c == 0), stop=(c == NC - 1),
                )
            vtmp = vtmp_p.tile([H, CH], BF16)
            nc.vector.tensor_copy(vtmp, v_ps[0:H, :])
            vt_ps = tp_ps.tile([P, TPC * H], BF16, tag="tp")
            for j in range(TPC):
                nc.tensor.transpose(
                    vt_ps[:, j * H : (j + 1) * H],
                    vtmp[:, j * P : (j + 1) * P],
                    ident[0:H, 0:H],
                )
            nc.vector.tensor_copy(
                vaug[:, TPC * ch : TPC * ch + TPC, 0:H],
                vt_ps.rearrange("p (j h) -> p j h", j=TPC),
            )

            # ---- attention: scores vs all causal key tiles, exp, PV ----
            smax = TPC * ch + TPC - 1
            o_ps = o_ps_p.tile([VA, CH], F32, tag="o")
            prev = None
            for s in range(smax + 1):
                diag = s >= TPC * ch
                col0 = (s - TPC * ch) * P if diag else 0
                wei = wei_ps.tile([P, CH], F32, tag="wei")
                nc.tensor.matmul(
                    wei[:, col0:],
                    kTlo[:, s * P : (s + 1) * P],
                    qkT[0:H, ch * CH + col0 : (ch + 1) * CH],
                    start=True,
                    stop=True,
                )
                pT = pt_p.tile([P, CH], BF16)
                nc.scalar.activation(pT[:, col0:], wei[:, col0:], Exp, scale=float(H) ** -0.5)
                if diag:
                    nc.vector.tensor_mul(
                        pT[:, col0 : col0 + P], pT[:, col0 : col0 + P], tri
                    )
                if prev is not None:
                    pcol0, ppT, ps = prev
                    nc.tensor.matmul(
                        o_ps[:, pcol0:], vaug[:, ps, :], ppT[:, pcol0:],
                        start=(ps == 0), stop=False,
                    )
                prev = (col0, pT, s)
            pcol0, ppT, ps = prev
            nc.tensor.matmul(
                o_ps[:, pcol0:], vaug[:, ps, :], ppT[:, pcol0:],
                start=(ps == 0), stop=True,
            )

            # ---- epilogue: transpose back, normalize, store ----
            osb = osb_p.tile([VA, CH], BF16)
            nc.vector.tensor_copy(osb, o_ps)
            ot_ps = tp_ps.tile([P, TPC * VA], BF16, tag="tp")
            for j in range(TPC):
                nc.tensor.transpose(
                    ot_ps[:, j * VA : (j + 1) * VA],
                    osb[:, j * P : (j + 1) * P],
                    ident[0:VA, 0:VA],
                )
            ost = ost_p.tile([P, TPC, H], F32)
            for j in range(TPC):
                rc = rc_p.tile([P, 1], F32)
                nc.vector.reciprocal(rc, ot_ps[:, j * VA + H : j * VA + H + 1])
                nc.vector.tensor_scalar_mul(
                    ost[:, j, :], ot_ps[:, j * VA : j * VA + H], rc
                )
            nc.sync.dma_start(
                outd[ch * CH : (ch + 1) * CH, :].rearrange("(n p) h -> p n h", p=P), ost
            )

    nc.compile()
    return nc


_NC = None


def _pack_weights(Wq, Wk, Wv):
    # [C, H] -> stationary tiles [P, NC, ...]: wqk[p, c, 0:64]=Wq[c*128+p],
    # wqk[p, c, 64:128]=Wk[c*128+p]; wv[p, c, :]=Wv[c*128+p]
    wq = Wq.reshape(NC, P, H)
    wk = Wk.reshape(NC, P, H)
    wqk = np.concatenate([wq, wk], axis=2).transpose(1, 0, 2)  # [P, NC, 128]
    wv = Wv.reshape(NC, P, H).transpose(1, 0, 2)  # [P, NC, 64]
    return (
        np.ascontiguousarray(wqk).astype(BF16_NP),
        np.ascontiguousarray(wv).astype(BF16_NP),
    )


def kernel(x, Wq, Wk, Wv, **run_kwargs):
    global _NC
    if _NC is None:
        _NC = build_kernel()
    x = np.asarray(x, dtype=np.float32)
    Wq = np.asarray(Wq, dtype=np.float32)
    Wk = np.asarray(Wk, dtype=np.float32)
    Wv = np.asarray(Wv, dtype=np.float32)
    wqk, wv = _pack_weights(Wq, Wk, Wv)
    xT = np.ascontiguousarray(x.transpose(0, 2, 1)).astype(BF16_NP)  # [B, C, T]
    in_maps = [{"xT": xT[b], "wqk": wqk, "wv": wv} for b in range(B)]
    res = run_bass_kernel_spmd(_NC, in_maps, core_ids=list(range(B)), **run_kwargs)
    out = np.stack([res.results[b]["out"] for b in range(B)])
    if run_kwargs:
        kernel.last_result = res
    return out


if __name__ == "__main__":
    rng = np.random.default_rng(0)
    ins = {
        "x": rng.standard_normal((B, T, C), dtype=np.float32),
        "Wq": rng.standard_normal((C, H), dtype=np.float32) / np.sqrt(C),
        "Wk": rng.standard_normal((C, H), dtype=np.float32) / np.sqrt(C),
        "Wv": rng.standard_normal((C, H), dtype=np.float32) / np.sqrt(C),
    }
    out = kernel(**ins)
    print("out", out.shape, out.dtype)


# revision 5
# speedup vs baseline: 1.4548x; 1.0003x over previous
"""Single-head causal attention on 8 NeuronCores (batch-parallel).

x [8, 2048, 1024], Wq/Wk/Wv [1024, 64] -> out [8, 2048, 64].
Each core handles one batch element.

v3: host-side layout prep + overlap-focused schedule.
  - x uploaded pre-transposed (xT [C, T]) in bf16: no on-chip transposes
    of x, half the DMA bytes.  [Wq|Wk] / Wv host-packed per c-tile.
  - All PE operands bf16 (FWL weight loads), f32 PSUM accumulation.
  - Attention runs a depth-2 software pipeline (scores lead PV by two
    groups) so PE never stalls on ACT's exp.
  - Projections for chunk ch+1 are interleaved into chunk ch's attention
    loop so ACT never waits for scores at chunk transitions.
  - Off-diagonal score tiles are processed in pairs sharing a [128,1024]
    PSUM tile (2 banks) -> one exp instruction per pair.
  qkT = [Wq|Wk].T @ xT      (per 512-col chunk; q rows 0:64, k 64:128)
  weiT[s,t] = k[s]·q[t]; pT = exp(weiT/8)   (no max-subtraction)
  outT_aug = [v|1|0pad].T @ pT  (ones column gives softmax denominators)
  out[t,h] = outT_aug[h,t] / outT_aug[64,t]
"""

from contextlib import ExitStack

import numpy as np
import ml_dtypes

import concourse.bass as bass
import concourse.mybir as mybir
import concourse.tile as tile
from concourse import bacc
from concourse.bass_utils import run_bass_kernel_spmd
from concourse.masks import make_identity, make_upper_triangular

B, T, C, H = 8, 2048, 1024, 64
P = 128                      # partition tile
NT = T // P                  # 16 row tiles
NC = C // P                  # 8 contraction tiles
CH = 512                     # t-chunk width (psum bank)
NCH = T // CH                # 4 chunks
TPC = CH // P                # 4 t-tiles per chunk
VA = 96                      # [v | 1 | 0] width for the output transpose
VP = 128                     # padded stationary width (FWL needs 128 cols)

BF16 = mybir.dt.bfloat16
F32 = mybir.dt.float32

Exp = mybir.ActivationFunctionType.Exp

BF16_NP = ml_dtypes.bfloat16


def build_kernel():
    nc = bacc.Bacc(
        "TRN2",
        target_bir_lowering=False,
        debug=False,
        enable_asserts=False,
        num_devices=B,
    )
    xtd = nc.dram_tensor("xT", [C, T], BF16, kind="ExternalInput").ap()
    wqkd = nc.dram_tensor("wqk", [P, NC, P], BF16, kind="ExternalInput").ap()
    wvd = nc.dram_tensor("wv", [P, NC, H], BF16, kind="ExternalInput").ap()
    outd = nc.dram_tensor("out", [T, H], F32, kind="ExternalOutput").ap()

    with tile.TileContext(nc) as tc, ExitStack() as ctx:
        const = ctx.enter_context(tc.tile_pool(name="const", bufs=1))
        persist = ctx.enter_context(tc.tile_pool(name="persist", bufs=1))
        vtmp_p = ctx.enter_context(tc.tile_pool(name="vtmp", bufs=2))
        pt_p = ctx.enter_context(tc.tile_pool(name="pt", bufs=4))
        osb_p = ctx.enter_context(tc.tile_pool(name="osb", bufs=2))
        ost_p = ctx.enter_context(tc.tile_pool(name="ost", bufs=2))
        rc_p = ctx.enter_context(tc.tile_pool(name="rc", bufs=4))
        # PSUM: scratch (proj accum + small transposes) 2 banks,
        # wei 2x[128,1024] = 4 banks, o 2x[128,512] = 2 banks -> 8 total
        scr_ps = ctx.enter_context(tc.tile_pool(name="scrps", bufs=2, space="PSUM"))
        wei_ps = ctx.enter_context(tc.tile_pool(name="weips", bufs=2, space="PSUM"))
        o_ps_p = ctx.enter_context(tc.tile_pool(name="ops", bufs=2, space="PSUM"))

        # x chunk 0 first (c-halves so projections start on the first half),
        # then weights, then the rest of x
        xTs = persist.tile([P, NC, T], BF16, tag="xTs")  # x.T: [c, t]
        xsrc = xtd.rearrange("(c p) t -> p c t", p=P)
        nc.sync.dma_start(xTs[:, 0:4, 0:CH], xsrc[:, 0:4, 0:CH])
        nc.sync.dma_start(xTs[:, 4:NC, 0:CH], xsrc[:, 4:NC, 0:CH])
        wqk = const.tile([P, NC, P], BF16, tag="wqk")
        nc.sync.dma_start(wqk, wqkd)
        wv = const.tile([P, NC, H], BF16, tag="wv")
        nc.sync.dma_start(wv, wvd)
        nc.sync.dma_start(xTs[:, :, CH : 2 * CH], xsrc[:, :, CH : 2 * CH])
        nc.sync.dma_start(xTs[:, :, 2 * CH : T], xsrc[:, :, 2 * CH : T])

        # masks: gpsimd builders write f32; DVE copy casts to bf16
        scr_i = const.tile([P, P], F32, tag="scr_i")
        make_identity(nc, scr_i)
        ident = const.tile([P, P], BF16, tag="ident")
        nc.vector.tensor_copy(ident, scr_i)
        scr_t = const.tile([P, P], F32, tag="scr_t")
        make_upper_triangular(nc, scr_t, val=1.0, diag=True)
        tri = const.tile([P, P], BF16, tag="tri")  # tri[p,j]=1 iff j>=p
        nc.vector.tensor_copy(tri, scr_t)

        qkT = persist.tile([P, T], BF16, tag="qkT")  # qT rows 0:64, kT 64:128
        kTlo = persist.tile([H, T], BF16, tag="kTlo")  # kT at partitions 0:64
        vaug = persist.tile([P, NT, VP], BF16, tag="vaug")  # [v | 1 | 0pad]
        ones = nc.const_aps.scalar_like(1.0, vaug)
        nc.vector.tensor_copy(vaug[:, :, H : H + 1], ones.broadcast_to((P, NT, 1)))
        zeros = nc.const_aps.scalar_like(0.0, vaug)
        nc.vector.tensor_copy(
            vaug[:, :, H + 1 : VP], zeros.broadcast_to((P, NT, VP - H - 1))
        )

        def proj_ops(ch):
            """Projection + v-prep for chunk ch as a list of thunks to
            interleave into the previous chunk's attention loop."""
            chs = slice(ch * CH, (ch + 1) * CH)
            ops = []
            state = {}

            def qk_mm(c):
                def f():
                    if c == 0:
                        state["qk"] = scr_ps.tile([P, CH], F32, tag="scr", name="qk_ps")
                    nc.tensor.matmul(
                        state["qk"], wqk[:, c, :], xTs[:, c, chs],
                        start=(c == 0), stop=(c == NC - 1),
                    )
                return f

            def qk_out():
                nc.vector.tensor_copy(qkT[:, chs], state["qk"])
                nc.sync.dma_start(kTlo[:, chs], qkT[H:P, chs])

            def v_mm(c):
                def f():
                    if c == 0:
                        state["v"] = scr_ps.tile([P, CH], F32, tag="scr", name="v_ps")
                    nc.tensor.matmul(
                        state["v"][0:H, :], wv[:, c, :], xTs[:, c, chs],
                        start=(c == 0), stop=(c == NC - 1),
                    )
                return f

            def v_out():
                vtmp = vtmp_p.tile([H, CH], BF16)
                nc.vector.tensor_copy(vtmp, state["v"][0:H, :])
                state["vtmp"] = vtmp

            def v_tp(j):
                def f():
                    if j == 0:
                        state["vt"] = scr_ps.tile([P, TPC * H], BF16, tag="scr", name="vt_ps")
                    nc.tensor.transpose(
                        state["vt"][:, j * H : (j + 1) * H],
                        state["vtmp"][:, j * P : (j + 1) * P],
                        ident[0:H, 0:H],
                    )
                return f

            def v_aug():
                nc.vector.tensor_copy(
                    vaug[:, TPC * ch : TPC * ch + TPC, 0:H],
                    state["vt"].rearrange("p (j h) -> p j h", j=TPC),
                )

            for c in range(NC):
                ops.append(qk_mm(c))
            ops.append(qk_out)
            for c in range(NC):
                ops.append(v_mm(c))
            ops.append(v_out)
            for j in range(TPC):
                ops.append(v_tp(j))
            ops.append(v_aug)
            return ops

        def run_ops(ops, n):
            for _ in range(n):
                if ops:
                    ops.pop(0)()

        # chunk 0 projections run up-front
        for op in proj_ops(0):
            op()

        for ch in range(NCH):
            chs0 = ch * CH
            inject = proj_ops(ch + 1) if ch + 1 < NCH else []

            # attention groups: off-diag s-pairs, then 4 diag singles
            groups = []
            for i in range(2 * ch):
                groups.append(("pair", 2 * i))
            for j in range(TPC):
                groups.append(("diag", TPC * ch + j))
            ngroups = len(groups)
            per_iter = -(-len(inject) // ngroups) if inject else 0  # ceil

            o_ps = o_ps_p.tile([P, CH], F32, tag="o")
            pipe = []  # emitted (kind, sfirst, pT, col0) awaiting PV

            def emit_pv(entry, stop):
                kind, sfirst, pT, col0 = entry
                if kind == "pair":
                    nc.tensor.matmul(
                        o_ps, vaug[:, sfirst, :], pT[:, 0:CH],
                        start=(sfirst == 0), stop=False,
                    )
                    nc.tensor.matmul(
                        o_ps, vaug[:, sfirst + 1, :], pT[:, CH : 2 * CH],
                        start=False, stop=stop,
                    )
                else:
                    nc.tensor.matmul(
                        o_ps[:, col0:], vaug[:, sfirst, :], pT[:, col0:CH],
                        start=(sfirst == 0), stop=stop,
                    )

            for gi, (kind, sfirst) in enumerate(groups):
                if kind == "pair":
                    wei = wei_ps.tile([P, 2 * CH], F32, tag="wei")
                    for u in range(2):
                        s = sfirst + u
                        nc.tensor.matmul(
                            wei[:, u * CH : (u + 1) * CH],
                            kTlo[:, s * P : (s + 1) * P],
                            qkT[0:H, chs0 : chs0 + CH],
                            start=True, stop=True,
                        )
                    pT = pt_p.tile([P, 2 * CH], BF16)
                    nc.scalar.activation(pT, wei, Exp, scale=float(H) ** -0.5)
                    pipe.append(("pair", sfirst, pT, 0))
                else:
                    s = sfirst
                    col0 = (s - TPC * ch) * P
                    wei = wei_ps.tile([P, 2 * CH], F32, tag="wei")
                    nc.tensor.matmul(
                        wei[:, col0:CH],
                        kTlo[:, s * P : (s + 1) * P],
                        qkT[0:H, chs0 + col0 : chs0 + CH],
                        start=True, stop=True,
                    )
                    pT = pt_p.tile([P, CH], BF16)
                    nc.scalar.activation(
                        pT[:, col0:], wei[:, col0:CH], Exp, scale=float(H) ** -0.5
                    )
                    nc.vector.tensor_mul(
                        pT[:, col0 : col0 + P], pT[:, col0 : col0 + P], tri
                    )
                    pipe.append(("diag", s, pT, col0))

                # depth-2: retire the group emitted two iterations ago
                if len(pipe) > 2:
                    emit_pv(pipe.pop(0), stop=False)
                run_ops(inject, per_iter)

            run_ops(inject, len(inject))
            while pipe:
                entry = pipe.pop(0)
                emit_pv(entry, stop=(len(pipe) == 0))

            # ---- epilogue: transpose back, normalize, store ----
            osb = osb_p.tile([VA, CH], BF16)
            ot_ps = scr_ps.tile([P, TPC * VA], BF16, tag="scr")
            for j in range(TPC):
                nc.vector.tensor_copy(
                    osb[:, j * P : (j + 1) * P], o_ps[0:VA, j * P : (j + 1) * P]
                )
                nc.tensor.transpose(
                    ot_ps[:, j * VA : (j + 1) * VA],
                    osb[:, j * P : (j + 1) * P],
                    ident[0:VA, 0:VA],
                )
            otv = ot_ps.rearrange("p (j v) -> p j v", j=TPC)
            rc = rc_p.tile([P, TPC, 1], F32)
            nc.vector.reciprocal(rc, otv[:, :, H : H + 1])
            ost = ost_p.tile([P, TPC, H], F32)
            nc.vector.tensor_mul(
                ost, otv[:, :, 0:H], rc.broadcast_to((P, TPC, H))
            )
            nc.sync.dma_start(
                outd[ch * CH : (ch + 1) * CH, :].rearrange("(n p) h -> p n h", p=P),
                ost,
            )

    nc.compile()
    return nc


_NC = None


def _pack_weights(Wq, Wk, Wv):
    # [C, H] -> stationary tiles [P, NC, ...]: wqk[p, c, 0:64]=Wq[c*128+p],
    # wqk[p, c, 64:128]=Wk[c*128+p]; wv[p, c, :]=Wv[c*128+p]
    wq = Wq.reshape(NC, P, H)
    wk = Wk.reshape(NC, P, H)
    wqk = np.concatenate([wq, wk], axis=2).transpose(1, 0, 2)  # [P, NC, 128]
    wv = Wv.reshape(NC, P, H).transpose(1, 0, 2)  # [P, NC, 64]
    return (
        np.ascontiguousarray(wqk).astype(BF16_NP),
        np.ascontiguousarray(wv).astype(BF16_NP),
    )


def kernel(x, Wq, Wk, Wv, **run_kwargs):
    global _NC
    if _NC is None:
        _NC = build_kernel()
    x = np.asarray(x, dtype=np.float32)
    Wq = np.asarray(Wq, dtype=np.float32)
    Wk = np.asarray(Wk, dtype=np.float32)
    Wv = np.asarray(Wv, dtype=np.float32)
    wqk, wv = _pack_weights(Wq, Wk, Wv)
    xT = np.ascontiguousarray(x.transpose(0, 2, 1)).astype(BF16_NP)  # [B, C, T]
    in_maps = [{"xT": xT[b], "wqk": wqk, "wv": wv} for b in range(B)]
    res = run_bass_kernel_spmd(_NC, in_maps, core_ids=list(range(B)), **run_kwargs)
    out = np.stack([res.results[b]["out"] for b in range(B)])
    if run_kwargs:
        kernel.last_result = res
    return out


if __name__ == "__main__":
    rng = np.random.default_rng(0)
    ins = {
        "x": rng.standard_normal((B, T, C), dtype=np.float32),
        "Wq": rng.standard_normal((C, H), dtype=np.float32) / np.sqrt(C),
        "Wk": rng.standard_normal((C, H), dtype=np.float32) / np.sqrt(C),
        "Wv": rng.standard_normal((C, H), dtype=np.float32) / np.sqrt(C),
    }
    out = kernel(**ins)
    print("out", out.shape, out.dtype)


# revision 6
# speedup vs baseline: 1.5539x; 1.0681x over previous
"""Single-head causal attention on 8 NeuronCores (batch-parallel).

x [8, 2048, 1024], Wq/Wk/Wv [1024, 64] -> out [8, 2048, 64].
Each core handles one batch element.

v3: host-side layout prep + overlap-focused schedule.
  - x uploaded pre-transposed (xT [C, T]) in bf16: no on-chip transposes
    of x, half the DMA bytes.  [Wq|Wk] / Wv host-packed per c-tile.
  - All PE operands bf16 (FWL weight loads), f32 PSUM accumulation.
  - Attention runs a depth-2 software pipeline (scores lead PV by two
    groups) so PE never stalls on ACT's exp.
  - Projections for chunk ch+1 are interleaved into chunk ch's attention
    loop so ACT never waits for scores at chunk transitions.
  - Off-diagonal score tiles are processed in pairs sharing a [128,1024]
    PSUM tile (2 banks) -> one exp instruction per pair.
  qkT = [Wq|Wk].T @ xT      (per 512-col chunk; q rows 0:64, k 64:128)
  weiT[s,t] = k[s]·q[t]; pT = exp(weiT/8)   (no max-subtraction)
  outT_aug = [v|1|0pad].T @ pT  (ones column gives softmax denominators)
  out[t,h] = outT_aug[h,t] / outT_aug[64,t]
"""

from contextlib import ExitStack

import numpy as np
import ml_dtypes

import concourse.bass as bass
import concourse.mybir as mybir
import concourse.tile as tile
from concourse import bacc
from concourse.bass_utils import run_bass_kernel_spmd
from concourse.masks import make_identity, make_upper_triangular

B, T, C, H = 8, 2048, 1024, 64
P = 128                      # partition tile
NT = T // P                  # 16 row tiles
NC = C // P                  # 8 contraction tiles
CH = 512                     # t-chunk width (psum bank)
NCH = T // CH                # 4 chunks
TPC = CH // P                # 4 t-tiles per chunk
VA = 96                      # [v | 1 | 0] width for the output transpose
VP = 128                     # padded stationary width (FWL needs 128 cols)

BF16 = mybir.dt.bfloat16
F32 = mybir.dt.float32

Exp = mybir.ActivationFunctionType.Exp

BF16_NP = ml_dtypes.bfloat16


def build_kernel():
    nc = bacc.Bacc(
        "TRN2",
        target_bir_lowering=False,
        debug=False,
        enable_asserts=False,
        num_devices=B,
    )
    xtd = nc.dram_tensor("xT", [P, NCH, NC, CH], BF16, kind="ExternalInput").ap()
    wqkd = nc.dram_tensor("wqk", [P, NC, P], BF16, kind="ExternalInput").ap()
    wvd = nc.dram_tensor("wv", [P, NC, H], BF16, kind="ExternalInput").ap()
    outd = nc.dram_tensor("out", [T, H], F32, kind="ExternalOutput").ap()

    with tile.TileContext(nc) as tc, ExitStack() as ctx:
        const = ctx.enter_context(tc.tile_pool(name="const", bufs=1))
        persist = ctx.enter_context(tc.tile_pool(name="persist", bufs=1))
        vtmp_p = ctx.enter_context(tc.tile_pool(name="vtmp", bufs=2))
        pt_p = ctx.enter_context(tc.tile_pool(name="pt", bufs=4))
        osb_p = ctx.enter_context(tc.tile_pool(name="osb", bufs=2))
        ost_p = ctx.enter_context(tc.tile_pool(name="ost", bufs=2))
        rc_p = ctx.enter_context(tc.tile_pool(name="rc", bufs=4))
        # PSUM: scratch (proj accum + small transposes) 2 banks,
        # wei 2x[128,1024] = 4 banks, o 2x[128,512] = 2 banks -> 8 total
        scr_ps = ctx.enter_context(tc.tile_pool(name="scrps", bufs=2, space="PSUM"))
        wei_ps = ctx.enter_context(tc.tile_pool(name="weips", bufs=2, space="PSUM"))
        o_ps_p = ctx.enter_context(tc.tile_pool(name="ops", bufs=2, space="PSUM"))

        # weights first (tiny), then x chunk-by-chunk; the chunk-major host
        # layout gives 4-8KB contiguous runs per partition per chunk
        wqk = const.tile([P, NC, P], BF16, tag="wqk")
        nc.sync.dma_start(wqk, wqkd)
        wv = const.tile([P, NC, H], BF16, tag="wv")
        nc.sync.dma_start(wv, wvd)
        xTs = persist.tile([P, NCH, NC, CH], BF16, tag="xTs")  # x.T per chunk
        nc.sync.dma_start(xTs[:, 0, 0:4], xtd[:, 0, 0:4])
        nc.sync.dma_start(xTs[:, 0, 4:NC], xtd[:, 0, 4:NC])
        for _c in range(1, NCH):
            nc.sync.dma_start(xTs[:, _c], xtd[:, _c])

        # masks: gpsimd builders write f32; DVE copy casts to bf16
        scr_i = const.tile([P, P], F32, tag="scr_i")
        make_identity(nc, scr_i)
        ident = const.tile([P, P], BF16, tag="ident")
        nc.vector.tensor_copy(ident, scr_i)
        scr_t = const.tile([P, P], F32, tag="scr_t")
        make_upper_triangular(nc, scr_t, val=1.0, diag=True)
        tri = const.tile([P, P], BF16, tag="tri")  # tri[p,j]=1 iff j>=p
        nc.vector.tensor_copy(tri, scr_t)

        qkT = persist.tile([P, T], BF16, tag="qkT")  # qT rows 0:64, kT 64:128
        kq = persist.tile([P, T], BF16, tag="kq")  # kT rows 0:64, qT 64:128
        vaug = persist.tile([P, NT, VP], BF16, tag="vaug")  # [v | 1 | 0pad]
        ones = nc.const_aps.scalar_like(1.0, vaug)
        nc.vector.tensor_copy(vaug[:, :, H : H + 1], ones.broadcast_to((P, NT, 1)))
        zeros = nc.const_aps.scalar_like(0.0, vaug)
        nc.vector.tensor_copy(
            vaug[:, :, H + 1 : VP], zeros.broadcast_to((P, NT, VP - H - 1))
        )

        def proj_ops(ch):
            """Projection + v-prep for chunk ch as a list of thunks to
            interleave into the previous chunk's attention loop."""
            chs = slice(ch * CH, (ch + 1) * CH)
            ops = []
            state = {}

            def qk_mm(c):
                def f():
                    if c == 0:
                        state["qk"] = scr_ps.tile([P, CH], F32, tag="scr", name="qk_ps")
                    nc.tensor.matmul(
                        state["qk"], wqk[:, c, :], xTs[:, ch, c, :],
                        start=(c == 0), stop=(c == NC - 1),
                    )
                return f

            def qk_out():
                nc.vector.tensor_copy(qkT[:, chs], state["qk"])
                nc.sync.dma_start(kq[0:H, chs], qkT[H:P, chs])
                nc.sync.dma_start(kq[H:P, chs], qkT[0:H, chs])

            def v_mm(c):
                def f():
                    if c == 0:
                        state["v"] = scr_ps.tile([P, CH], F32, tag="scr", name="v_ps")
                    nc.tensor.matmul(
                        state["v"][0:H, :], wv[:, c, :], xTs[:, ch, c, :],
                        start=(c == 0), stop=(c == NC - 1),
                    )
                return f

            def v_out():
                vtmp = vtmp_p.tile([H, CH], BF16)
                nc.vector.tensor_copy(vtmp, state["v"][0:H, :])
                state["vtmp"] = vtmp

            def v_tp(j):
                def f():
                    if j == 0:
                        state["vt"] = scr_ps.tile([P, TPC * H], BF16, tag="scr", name="vt_ps")
                    nc.tensor.transpose(
                        state["vt"][:, j * H : (j + 1) * H],
                        state["vtmp"][:, j * P : (j + 1) * P],
                        ident[0:H, 0:H],
                    )
                return f

            def v_aug():
                nc.vector.tensor_copy(
                    vaug[:, TPC * ch : TPC * ch + TPC, 0:H],
                    state["vt"].rearrange("p (j h) -> p j h", j=TPC),
                )

            for c in range(NC):
                ops.append(qk_mm(c))
            ops.append(qk_out)
            for c in range(NC):
                ops.append(v_mm(c))
            ops.append(v_out)
            for j in range(TPC):
                ops.append(v_tp(j))
            ops.append(v_aug)
            return ops

        def run_ops(ops, n):
            for _ in range(n):
                if ops:
                    ops.pop(0)()

        # chunk 0 projections run up-front
        for op in proj_ops(0):
            op()

        for ch in range(NCH):
            chs0 = ch * CH
            inject = proj_ops(ch + 1) if ch + 1 < NCH else []

            # attention groups: off-diag s-pairs, then 4 diag singles
            groups = []
            for i in range(2 * ch):
                groups.append(("pair", 2 * i))
            for j in range(TPC):
                groups.append(("diag", TPC * ch + j))
            ngroups = len(groups)
            per_iter = -(-len(inject) // ngroups) if inject else 0  # ceil

            o_ps = o_ps_p.tile([P, CH], F32, tag="o")
            pipe = []  # emitted (kind, sfirst, pT, col0) awaiting PV

            def emit_pv(entry, stop):
                kind, sfirst, pT, col0 = entry
                if kind == "pair":
                    nc.tensor.matmul(
                        o_ps, vaug[:, sfirst, :], pT[:, 0:CH],
                        start=(sfirst == 0), stop=False,
                    )
                    nc.tensor.matmul(
                        o_ps, vaug[:, sfirst + 1, :], pT[:, CH : 2 * CH],
                        start=False, stop=stop,
                    )
                else:
                    nc.tensor.matmul(
                        o_ps[:, col0:], vaug[:, sfirst, :], pT[:, col0:CH],
                        start=(sfirst == 0), stop=stop,
                    )

            for gi, (kind, sfirst) in enumerate(groups):
                if kind == "pair":
                    wei = wei_ps.tile([P, 2 * CH], F32, tag="wei")
                    s = sfirst
                    nc.tensor.matmul(
                        wei[:, 0:CH],
                        kq[0:H, s * P : (s + 1) * P],
                        qkT[0:H, chs0 : chs0 + CH],
                        start=True, stop=True,
                    )
                    nc.tensor.matmul(
                        wei[:, CH : 2 * CH],
                        qkT[H:P, (s + 1) * P : (s + 2) * P],
                        kq[H:P, chs0 : chs0 + CH],
                        start=True, stop=True,
                    )
                    pT = pt_p.tile([P, 2 * CH], BF16)
                    nc.scalar.activation(pT, wei, Exp, scale=float(H) ** -0.5)
                    pipe.append(("pair", sfirst, pT, 0))
                else:
                    s = sfirst
                    col0 = (s - TPC * ch) * P
                    wei = wei_ps.tile([P, 2 * CH], F32, tag="wei")
                    nc.tensor.matmul(
                        wei[:, col0:CH],
                        kq[0:H, s * P : (s + 1) * P],
                        qkT[0:H, chs0 + col0 : chs0 + CH],
                        start=True, stop=True,
                    )
                    pT = pt_p.tile([P, CH], BF16)
                    nc.scalar.activation(
                        pT[:, col0:], wei[:, col0:CH], Exp, scale=float(H) ** -0.5
                    )
                    nc.vector.tensor_mul(
                        pT[:, col0 : col0 + P], pT[:, col0 : col0 + P], tri
                    )
                    pipe.append(("diag", s, pT, col0))

                # depth-2: retire the group emitted two iterations ago
                if len(pipe) > 2:
                    emit_pv(pipe.pop(0), stop=False)
                run_ops(inject, per_iter)

            run_ops(inject, len(inject))
            while pipe:
                entry = pipe.pop(0)
                emit_pv(entry, stop=(len(pipe) == 0))

            # ---- epilogue: transpose back, normalize, store ----
            osb = osb_p.tile([VA, CH], BF16)
            ot_ps = scr_ps.tile([P, TPC * VA], BF16, tag="scr")
            for j in range(TPC):
                nc.vector.tensor_copy(
                    osb[:, j * P : (j + 1) * P], o_ps[0:VA, j * P : (j + 1) * P]
                )
                nc.tensor.transpose(
                    ot_ps[:, j * VA : (j + 1) * VA],
                    osb[:, j * P : (j + 1) * P],
                    ident[0:VA, 0:VA],
                )
            otv = ot_ps.rearrange("p (j v) -> p j v", j=TPC)
            rc = rc_p.tile([P, TPC, 1], F32)
            nc.vector.reciprocal(rc, otv[:, :, H : H + 1])
            ost = ost_p.tile([P, TPC, H], F32)
            nc.vector.tensor_mul(
                ost, otv[:, :, 0:H], rc.broadcast_to((P, TPC, H))
            )
            nc.sync.dma_start(
                outd[ch * CH : (ch + 1) * CH, :].rearrange("(n p) h -> p n h", p=P),
                ost,
            )

    nc.compile()
    return nc


_NC = None


def _pack_weights(Wq, Wk, Wv):
    # [C, H] -> stationary tiles [P, NC, ...]: wqk[p, c, 0:64]=Wq[c*128+p],
    # wqk[p, c, 64:128]=Wk[c*128+p]; wv[p, c, :]=Wv[c*128+p]
    wq = Wq.reshape(NC, P, H)
    wk = Wk.reshape(NC, P, H)
    wqk = np.concatenate([wq, wk], axis=2).transpose(1, 0, 2)  # [P, NC, 128]
    wv = Wv.reshape(NC, P, H).transpose(1, 0, 2)  # [P, NC, 64]
    return (
        np.ascontiguousarray(wqk).astype(BF16_NP),
        np.ascontiguousarray(wv).astype(BF16_NP),
    )


def kernel(x, Wq, Wk, Wv, **run_kwargs):
    global _NC
    if _NC is None:
        _NC = build_kernel()
    x = np.asarray(x, dtype=np.float32)
    Wq = np.asarray(Wq, dtype=np.float32)
    Wk = np.asarray(Wk, dtype=np.float32)
    Wv = np.asarray(Wv, dtype=np.float32)
    wqk, wv = _pack_weights(Wq, Wk, Wv)
    # [B, C, T] -> chunk-major [B, P, NCH, NC, CH]
    xT = x.transpose(0, 2, 1).astype(BF16_NP)
    xTq = np.ascontiguousarray(
        xT.reshape(B, NC, P, NCH, CH).transpose(0, 2, 3, 1, 4)
    )
    in_maps = [{"xT": xTq[b], "wqk": wqk, "wv": wv} for b in range(B)]
    res = run_bass_kernel_spmd(_NC, in_maps, core_ids=list(range(B)), **run_kwargs)
    out = np.stack([res.results[b]["out"] for b in range(B)])
    if run_kwargs:
        kernel.last_result = res
    return out


if __name__ == "__main__":
    rng = np.random.default_rng(0)
    ins = {
        "x": rng.standard_normal((B, T, C), dtype=np.float32),
        "Wq": rng.standard_normal((C, H), dtype=np.float32) / np.sqrt(C),
        "Wk": rng.standard_normal((C, H), dtype=np.float32) / np.sqrt(C),
        "Wv": rng.standard_normal((C, H), dtype=np.float32) / np.sqrt(C),
    }
    out = kernel(**ins)
    print("out", out.shape, out.dtype)
